# revision 22
# baseline (speedup 1.0000x reference)
"""Trainium2 Bass kernel for nn_Decoder (worker/task label-probability decoder).

Math:
    worker_feature = inputs[:2048, :64]          # [Wn, A]
    tau            = inputs[2048:, :16]          # [T, L]
    p1 = sigmoid(worker_feature @ W + b)         # [Wn, 1]
    p2 = (1 - p1) / (L - 1)
    P[i, j, l] = p1[i]^tau[j,l] * p2[i]^(1 - tau[j,l])
               = exp(a[i] * tau[j,l] + c[i]),  a = ln p1 - ln p2, c = ln p2

Sharding: pure data parallel over the worker axis (dim 0), 256 workers per
core across 8 cores; tau/W/b replicated. No communication.

Per-core device layout: workers on SBUF partitions (2 groups of 128), task
axis flattened on the free dimension. tau arrives striped [16, 2048] so the
load spreads over 16 DMA ports; the otherwise-idle GPSIMD engine then
replicates each stripe to all 128 partitions (partition_broadcast — an exact
fp32 copy). The scalar engine computes Exp(a*tau + c) in one pass with
per-partition scale/bias, and results stream to HBM as 2 MiB writes. The
only non-trivial HBM traffic is the 32 MiB output per core, so the kernel
runs at the DMA roofline.
"""

import numpy as np

try:
    import concourse.bass as bass  # noqa: F401
except ImportError:  # fall back to the container's repo checkout
    import sys

    for _p in ("/root/.axon_site/_ro/trn_rl_repo", "/opt/trn_rl_repo"):
        if _p not in sys.path:
            sys.path.append(_p)

import concourse.bass as bass
import concourse.tile as tile
from concourse import library_config, mybir
from concourse.bass_utils import run_bass_kernel_spmd
from concourse.vector_clock import ScopedClock

WN = 2048  # workers total
TN = 2048  # tasks
L = 16  # edge types / labels
A = 64  # ability features
NCORES = 8
WPC = WN // NCORES  # workers per core (256)
G = WPC // 128  # partition groups per core (2)
F = TN * L  # flattened task axis (32768)
CH = 2048  # tau stripe length (one partition_broadcast each)
NST = F // CH  # tau stripes (16)
OT = 4096  # free-dim elements per output tile / ACT op (2 MiB tiles)

_AF = mybir.ActivationFunctionType


class _TC(tile.TileContext):
    """TileContext legalized for a walrus that allows one sync-wait per inst.

    The walrus build in this container rejects any instruction carrying more
    than one sync-wait command. After Tile's normal scheduling + the exit
    drain/barrier, rewrite every multi-wait instruction into a chain of
    same-engine NOPs (one wait each) followed by the instruction with the
    final wait.
    """

    def _drain_and_barrier(self, tick_clock, wait_clock):
        super()._drain_and_barrier(tick_clock, wait_clock)
        self._split_multi_waits()

    def _fresh_nop(self, engine):
        inst = self.nc.engines[engine].nop(nofuse=True).ins
        self.nc.cur_bb.bb.instructions.remove(inst)
        return inst

    def _split_multi_waits(self):
        for fn in self.nc.m.functions:
            for bb in fn.blocks:
                snapshot = list(bb.instructions)
                if not any(
                    inst.sync_info and len(inst.sync_info.on_wait) > 1
                    for inst in snapshot
                ):
                    continue
                new = []
                for inst in snapshot:
                    si = inst.sync_info
                    if si is not None and si.on_wait and len(si.on_wait) > 1:
                        waits = list(si.on_wait)
                        si.on_wait = waits[-1:]
                        inst.sync_info = si
                        for wt in waits[:-1]:
                            nop = self._fresh_nop(inst.engine)
                            nop.sync_info = mybir.SyncInfo(on_wait=[wt], on_update=[])
                            new.append(nop)
                    new.append(inst)
                bb.instructions[:] = new


def build_nc():
    nc = bass.Bass("TRN2")
    wf = nc.dram_tensor("wf", [WPC, A], mybir.dt.float32, kind="ExternalInput")
    tau_in = nc.dram_tensor("tau", [NST, CH], mybir.dt.float32, kind="ExternalInput")
    w_in = nc.dram_tensor("W", [A], mybir.dt.float32, kind="ExternalInput")
    b_in = nc.dram_tensor("b", [1], mybir.dt.float32, kind="ExternalInput")
    out = nc.dram_tensor("out", [G, 128, F], mybir.dt.float32, kind="ExternalOutput")

    f32 = mybir.dt.float32

    with _TC(nc) as tc:
        with (
            tc.tile_pool(name="const", bufs=1) as const,
            tc.tile_pool(name="reps", bufs=2) as reps,
            tc.tile_pool(name="outs", bufs=3) as outs,
        ):
            # gpsimd ucode library containing partition_broadcast
            nc.gpsimd.load_library(library_config.mlp)

            # ---- constant / prep tiles ----
            wf_sb = const.tile([128, G, A], f32)
            nc.sync.dma_start(
                out=wf_sb, in_=wf[:].rearrange("(g p) a -> p g a", p=128)
            )

            w_ap = w_in[:]
            w_sb = const.tile([128, A], f32)
            nc.sync.dma_start(
                out=w_sb,
                in_=bass.AP(tensor=w_ap.tensor, offset=w_ap.offset, ap=[[0, 128], [1, A]]),
            )
            b_ap = b_in[:]
            b_sb = const.tile([128, 1], f32)
            nc.sync.dma_start(
                out=b_sb,
                in_=bass.AP(tensor=b_ap.tensor, offset=b_ap.offset, ap=[[0, 128], [1, 1]]),
            )

            # ---- per-worker scalars: a = ln p1 - ln p2, c = ln p2 ----
            x = const.tile([128, G], f32)
            for g in range(G):
                prod = const.tile([128, A], f32, tag=f"prod{g}")
                nc.vector.tensor_mul(prod, wf_sb[:, g, :], w_sb)
                nc.vector.reduce_sum(x[:, g : g + 1], prod, axis=mybir.AxisListType.X)

            bneg = const.tile([128, 1], f32)
            nc.vector.tensor_scalar_mul(bneg, b_sb, -1.0)
            # e = exp(-(x + b));  p1 = 1 / (1 + e)
            e = const.tile([128, G], f32)
            nc.scalar.activation(e, x, _AF.Exp, bias=bneg[:, 0:1], scale=-1.0)
            nc.vector.tensor_scalar_add(e, e, 1.0)
            p1 = const.tile([128, G], f32)
            nc.vector.reciprocal(p1, e)
            p2 = const.tile([128, G], f32)
            nc.vector.tensor_scalar(
                p2,
                p1,
                scalar1=-1.0 / (L - 1),
                scalar2=1.0 / (L - 1),
                op0=mybir.AluOpType.mult,
                op1=mybir.AluOpType.add,
            )
            lp1 = const.tile([128, G], f32)
            nc.scalar.activation(lp1, p1, _AF.Ln)
            lp2 = const.tile([128, G], f32)
            nc.scalar.activation(lp2, p2, _AF.Ln)
            a_sb = const.tile([128, G], f32)
            nc.vector.tensor_sub(a_sb, lp1, lp2)

            # ---- main loop: broadcast tau -> ACT exp -> DMA out ----
            tau_flat = tau_in[:].rearrange("s c -> (s c)")

            def emit_round(rep_ap, f0, sz, key):
                for g in range(G):
                    ot = outs.tile(
                        [128, sz], f32, tag=f"ot{g}", name=f"ot{g}_{key}", bufs=3
                    )
                    nc.scalar.activation(
                        ot,
                        rep_ap,
                        _AF.Exp,
                        bias=lp2[:, g : g + 1],
                        scale=a_sb[:, g : g + 1],
                    )
                    # Alternate the two HWDGE rings (SP / ACT) so ring-level
                    # dispatch overheads pipeline across consecutive writes.
                    eng = nc.sync if g == 0 else nc.scalar
                    eng.dma_start(out=out[g, :, f0 : f0 + sz], in_=ot)

            def hbm_rep(f0, sz, key):
                # Replicate straight from HBM — used only during the ramp,
                # while the HBM write stream is still idle.
                rep = reps.tile([128, sz], f32, tag="rep", name=f"rep_{key}", bufs=3)
                nc.gpsimd.dma_start(
                    out=rep,
                    in_=bass.AP(
                        tensor=tau_flat.tensor,
                        offset=tau_flat.offset + f0,
                        ap=[[0, 128], [1, sz]],
                    ),
                )
                return rep

            # tau stripes for the broadcast rounds. Rides the ACT HWDGE ring:
            # ACT is idle this early, and the transfer never queues behind
            # the output writes on SP's ring.
            tau_sb = const.tile([NST, CH], f32)
            nc.scalar.dma_start(out=tau_sb, in_=tau_in[:])

            # Round 0 in 1 MiB pieces from HBM (the write stream is idle, so
            # the 2 MiB broadcast read is free) so the first write launches
            # ASAP.
            for h in range(OT // CH):
                rep = hbm_rep(h * CH, CH, f"w0{h}")
                emit_round(rep, h * CH, CH, f"w0{h}")

            # Remaining rounds: partition_broadcast costs no HBM traffic. It
            # needs its source on partition 0, so first gather each round's
            # stripes into one row via a tiny SBUF->SBUF DMA on the Pool ring.
            def bcast_rep(q):
                stg = reps.tile([1, OT], f32, tag="stg", name="stg")
                nc.gpsimd.dma_start(
                    out=stg,
                    in_=tau_sb[q * (OT // CH) : (q + 1) * (OT // CH), :],
                )
                rep = reps.tile([128, OT], f32, tag="rep", name="rep", bufs=3)
                for h in range(OT // CH):
                    nc.gpsimd.partition_broadcast(
                        rep[:, h * CH : (h + 1) * CH],
                        stg[0:1, h * CH : (h + 1) * CH],
                    )
                return rep

            # Round 1 in 1 MiB pieces (shorter ACT->write latency while the
            # pipeline is still filling), the rest as 2 MiB rounds.
            rep = bcast_rep(1)
            for h in range(OT // CH):
                emit_round(rep[:, h * CH : (h + 1) * CH], OT + h * CH, CH, f"q1{h}")
            for q in range(2, F // OT):
                emit_round(bcast_rep(q), q * OT, OT, f"q{q}")
    return nc


_NC = None


def kernel(inputs, W, b, worker_num=WN, task_num=TN, edge_type=L, ability_num=A, **_kw):
    global _NC
    inputs = np.ascontiguousarray(np.asarray(inputs, dtype=np.float32))
    W = np.asarray(W, dtype=np.float32).reshape(A)
    b = np.asarray(b, dtype=np.float32).reshape(1)
    assert inputs.shape == (WN + TN, A)

    wf = inputs[:WN, :A]
    tau = np.ascontiguousarray(inputs[WN:, :L].reshape(NST, CH))

    if _NC is None:
        _NC = build_nc()

    in_maps = [
        {
            "wf": np.ascontiguousarray(wf[k * WPC : (k + 1) * WPC]),
            "tau": tau,
            "W": W,
            "b": b,
        }
        for k in range(NCORES)
    ]
    res = run_bass_kernel_spmd(_NC, in_maps, core_ids=list(range(NCORES)))
    parts = [r["out"].reshape(WPC, TN, L) for r in res.results]
    return np.concatenate(parts, axis=0)


# revision 23
# speedup vs baseline: 1.0408x; 1.0408x over previous
"""Trainium2 Bass kernel for nn_Decoder (worker/task label-probability decoder).

Math:
    worker_feature = inputs[:2048, :64]          # [Wn, A]
    tau            = inputs[2048:, :16]          # [T, L]
    p1 = sigmoid(worker_feature @ W + b)         # [Wn, 1]
    p2 = (1 - p1) / (L - 1)
    P[i, j, l] = p1[i]^tau[j,l] * p2[i]^(1 - tau[j,l])
               = exp(a[i] * tau[j,l] + c[i]),  a = ln p1 - ln p2, c = ln p2

Sharding: pure data parallel over the worker axis (dim 0), 256 workers per
core across 8 cores; tau/W/b replicated. No communication.

Per-core device layout: workers on SBUF partitions (2 groups of 128), task
axis flattened on the free dimension. tau arrives striped [16, 2048] so the
load spreads over 16 DMA ports; the otherwise-idle GPSIMD engine then
replicates each stripe to all 128 partitions (partition_broadcast — an exact
fp32 copy). The scalar engine computes Exp(a*tau + c) in one pass with
per-partition scale/bias, and results stream to HBM as 2 MiB writes. The
only non-trivial HBM traffic is the 32 MiB output per core, so the kernel
runs at the DMA roofline.
"""

import numpy as np

try:
    import concourse.bass as bass  # noqa: F401
except ImportError:  # fall back to the container's repo checkout
    import sys

    for _p in ("/root/.axon_site/_ro/trn_rl_repo", "/opt/trn_rl_repo"):
        if _p not in sys.path:
            sys.path.append(_p)

import concourse.bass as bass
import concourse.tile as tile
from concourse import library_config, mybir
from concourse.bass_utils import run_bass_kernel_spmd
from concourse.vector_clock import ScopedClock

WN = 2048  # workers total
TN = 2048  # tasks
L = 16  # edge types / labels
A = 64  # ability features
NCORES = 8
WPC = WN // NCORES  # workers per core (256)
G = WPC // 128  # partition groups per core (2)
F = TN * L  # flattened task axis (32768)
CH = 2048  # tau stripe length (one partition_broadcast each)
NST = F // CH  # tau stripes (16)
OT = 4096  # free-dim elements per output tile / ACT op (2 MiB tiles)

_AF = mybir.ActivationFunctionType


class _TC(tile.TileContext):
    """TileContext legalized for a walrus that allows one sync-wait per inst.

    The walrus build in this container rejects any instruction carrying more
    than one sync-wait command. After Tile's normal scheduling + the exit
    drain/barrier, rewrite every multi-wait instruction into a chain of
    same-engine NOPs (one wait each) followed by the instruction with the
    final wait.
    """

    def _drain_and_barrier(self, tick_clock, wait_clock):
        super()._drain_and_barrier(tick_clock, wait_clock)
        self._split_multi_waits()

    def _fresh_nop(self, engine):
        inst = self.nc.engines[engine].nop(nofuse=True).ins
        self.nc.cur_bb.bb.instructions.remove(inst)
        return inst

    def _split_multi_waits(self):
        for fn in self.nc.m.functions:
            for bb in fn.blocks:
                snapshot = list(bb.instructions)
                if not any(
                    inst.sync_info and len(inst.sync_info.on_wait) > 1
                    for inst in snapshot
                ):
                    continue
                new = []
                for inst in snapshot:
                    si = inst.sync_info
                    if si is not None and si.on_wait and len(si.on_wait) > 1:
                        waits = list(si.on_wait)
                        si.on_wait = waits[-1:]
                        inst.sync_info = si
                        for wt in waits[:-1]:
                            nop = self._fresh_nop(inst.engine)
                            nop.sync_info = mybir.SyncInfo(on_wait=[wt], on_update=[])
                            new.append(nop)
                    new.append(inst)
                bb.instructions[:] = new


def build_nc():
    nc = bass.Bass("TRN2")
    wf = nc.dram_tensor("wf", [WPC, A], mybir.dt.float32, kind="ExternalInput")
    tau_in = nc.dram_tensor("tau", [NST, CH], mybir.dt.float32, kind="ExternalInput")
    w_in = nc.dram_tensor("W", [A], mybir.dt.float32, kind="ExternalInput")
    b_in = nc.dram_tensor("b", [1], mybir.dt.float32, kind="ExternalInput")
    out = nc.dram_tensor("out", [G, 128, F], mybir.dt.float32, kind="ExternalOutput")

    f32 = mybir.dt.float32

    with _TC(nc) as tc:
        with (
            tc.tile_pool(name="const", bufs=1) as const,
            tc.tile_pool(name="reps", bufs=2) as reps,
            tc.tile_pool(name="outs", bufs=3) as outs,
        ):
            # gpsimd ucode library containing partition_broadcast
            nc.gpsimd.load_library(library_config.mlp)

            # ---- constant / prep tiles ----
            wf_sb = const.tile([128, G, A], f32)
            nc.sync.dma_start(
                out=wf_sb, in_=wf[:].rearrange("(g p) a -> p g a", p=128)
            )

            w_ap = w_in[:]
            w_sb = const.tile([128, A], f32)
            nc.sync.dma_start(
                out=w_sb,
                in_=bass.AP(tensor=w_ap.tensor, offset=w_ap.offset, ap=[[0, 128], [1, A]]),
            )
            b_ap = b_in[:]
            b_sb = const.tile([128, 1], f32)
            nc.sync.dma_start(
                out=b_sb,
                in_=bass.AP(tensor=b_ap.tensor, offset=b_ap.offset, ap=[[0, 128], [1, 1]]),
            )

            # ---- per-worker scalars: a = ln p1 - ln p2, c = ln p2 ----
            x = const.tile([128, G], f32)
            for g in range(G):
                prod = const.tile([128, A], f32, tag=f"prod{g}")
                nc.vector.tensor_mul(prod, wf_sb[:, g, :], w_sb)
                nc.vector.reduce_sum(x[:, g : g + 1], prod, axis=mybir.AxisListType.X)

            bneg = const.tile([128, 1], f32)
            nc.vector.tensor_scalar_mul(bneg, b_sb, -1.0)
            # e = exp(-(x + b));  p1 = 1 / (1 + e)
            e = const.tile([128, G], f32)
            nc.scalar.activation(e, x, _AF.Exp, bias=bneg[:, 0:1], scale=-1.0)
            nc.vector.tensor_scalar_add(e, e, 1.0)
            p1 = const.tile([128, G], f32)
            nc.vector.reciprocal(p1, e)
            p2 = const.tile([128, G], f32)
            nc.vector.tensor_scalar(
                p2,
                p1,
                scalar1=-1.0 / (L - 1),
                scalar2=1.0 / (L - 1),
                op0=mybir.AluOpType.mult,
                op1=mybir.AluOpType.add,
            )
            lp1 = const.tile([128, G], f32)
            nc.scalar.activation(lp1, p1, _AF.Ln)
            lp2 = const.tile([128, G], f32)
            nc.scalar.activation(lp2, p2, _AF.Ln)
            a_sb = const.tile([128, G], f32)
            nc.vector.tensor_sub(a_sb, lp1, lp2)

            # ---- main loop: broadcast tau -> ACT exp -> DMA out ----
            tau_flat = tau_in[:].rearrange("s c -> (s c)")

            def emit_round(rep_ap, f0, sz, key):
                for g in range(G):
                    ot = outs.tile(
                        [128, sz], f32, tag=f"ot{g}", name=f"ot{g}_{key}", bufs=3
                    )
                    nc.scalar.activation(
                        ot,
                        rep_ap,
                        _AF.Exp,
                        bias=lp2[:, g : g + 1],
                        scale=a_sb[:, g : g + 1],
                    )
                    nc.sync.dma_start(out=out[g, :, f0 : f0 + sz], in_=ot)

            def hbm_rep(f0, sz, key):
                # Replicate straight from HBM — used only during the ramp,
                # while the HBM write stream is still idle.
                rep = reps.tile([128, sz], f32, tag="rep", name=f"rep_{key}", bufs=3)
                nc.gpsimd.dma_start(
                    out=rep,
                    in_=bass.AP(
                        tensor=tau_flat.tensor,
                        offset=tau_flat.offset + f0,
                        ap=[[0, 128], [1, sz]],
                    ),
                )
                return rep

            # tau stripes for the broadcast rounds. Rides the ACT HWDGE ring:
            # ACT is idle this early, and the transfer never queues behind
            # the output writes on SP's ring.
            tau_sb = const.tile([NST, CH], f32)
            nc.scalar.dma_start(out=tau_sb, in_=tau_in[:])

            # Round 0 in 1 MiB pieces from HBM (the write stream is idle, so
            # the 2 MiB broadcast read is free) so the first write launches
            # ASAP.
            for h in range(OT // CH):
                rep = hbm_rep(h * CH, CH, f"w0{h}")
                emit_round(rep, h * CH, CH, f"w0{h}")

            # Remaining rounds: partition_broadcast costs no HBM traffic. It
            # needs its source on partition 0, so first gather each round's
            # stripes into one row via a tiny SBUF->SBUF DMA on the Pool ring.
            def bcast_rep(q):
                stg = reps.tile([1, OT], f32, tag="stg", name="stg")
                nc.gpsimd.dma_start(
                    out=stg,
                    in_=tau_sb[q * (OT // CH) : (q + 1) * (OT // CH), :],
                )
                rep = reps.tile([128, OT], f32, tag="rep", name="rep", bufs=3)
                for h in range(OT // CH):
                    nc.gpsimd.partition_broadcast(
                        rep[:, h * CH : (h + 1) * CH],
                        stg[0:1, h * CH : (h + 1) * CH],
                    )
                return rep

            # Round 1 in 1 MiB pieces (shorter ACT->write latency while the
            # pipeline is still filling), the rest as 2 MiB rounds.
            rep = bcast_rep(1)
            for h in range(OT // CH):
                emit_round(rep[:, h * CH : (h + 1) * CH], OT + h * CH, CH, f"q1{h}")
            for q in range(2, F // OT):
                emit_round(bcast_rep(q), q * OT, OT, f"q{q}")
    return nc


_NC = None


def kernel(inputs, W, b, worker_num=WN, task_num=TN, edge_type=L, ability_num=A, **_kw):
    global _NC
    inputs = np.ascontiguousarray(np.asarray(inputs, dtype=np.float32))
    W = np.asarray(W, dtype=np.float32).reshape(A)
    b = np.asarray(b, dtype=np.float32).reshape(1)
    assert inputs.shape == (WN + TN, A)

    wf = inputs[:WN, :A]
    tau = np.ascontiguousarray(inputs[WN:, :L].reshape(NST, CH))

    if _NC is None:
        _NC = build_nc()

    in_maps = [
        {
            "wf": np.ascontiguousarray(wf[k * WPC : (k + 1) * WPC]),
            "tau": tau,
            "W": W,
            "b": b,
        }
        for k in range(NCORES)
    ]
    res = run_bass_kernel_spmd(_NC, in_maps, core_ids=list(range(NCORES)))
    parts = [r["out"].reshape(WPC, TN, L) for r in res.results]
    return np.concatenate(parts, axis=0)
